# revision 28
# baseline (speedup 1.0000x reference)
"""IsoMaxPlus first-part kernel for TRN2 (8 NeuronCores, data-parallel on B).

out[b, c] = -|s| * sqrt(max(2 - 2 * <f_b/||f_b||, p_c/||p_c||>, 1e-12))

v3 strategy per core (B-shard of 8192 rows):
  prolog: load prototypes (host-padded to [1024, 512]) fp32, row-normalize
          and scale by -16 into bf16 (DVE), PE-transpose (identity matmul),
          DVE psum->SBUF copy casting to fp8e4: pnT8 [128, 4, 1024].
  main:   64 blocks of 128 feature rows:
          DMA f [128,512] fp32 -> DVE bf16 cast fb -> DVE
          tensor_tensor_reduce(fb*fb, row-accum) -> batched (per 4 blocks)
          ACT Sqrt + DVE recip + DVE mul give scale_a = s^2/128 * 16/||f||
          -> 4x PE bf16 transpose into half a PSUM bank -> DVE copy to SBUF
          fp8 -> 2 n-splits x 2 DoubleRow fp8 matmuls (dots = -16*||f||*t,
          fp32 psum) -> ACT Sqrt(dots*scale_a + 2s^2) psum->SBUF bf16 ->
          DVE negate -> DMA out bf16.
  host:   upcast bf16 -> fp32, concat shards.
Engine budget per block: DMA ~1.4us HBM / PE ~1.2-2us / ACT ~1.2us /
DVE ~1.4us -> target ~100-130us/core (vs 729us baseline).
"""

import numpy as np
from contextlib import ExitStack

import concourse.bass as bass
import concourse.tile as tile
from concourse import bacc, mybir
from concourse.bass import ts
from concourse.bass_utils import run_bass_kernel_spmd

N_CORES = 8
B, D, C = 65536, 512, 1000
CP = 1024                  # prototypes padded (zeros) to a multiple of 128
BS = B // N_CORES          # 8192 rows per core
NB = BS // 128             # 64 row blocks
KC = D // 128              # 4 contraction chunks
NSPLIT = ((0, 512), (512, 488))  # psum bank halves
F32 = mybir.dt.float32
BF16 = mybir.dt.bfloat16
F8 = mybir.dt.float8e4
DR = mybir.MatmulPerfMode.DoubleRow


def _emit(nc):
    f_dram = nc.dram_tensor("features", [BS, D], F32, kind="ExternalInput").ap()
    p_dram = nc.dram_tensor("prototypes", [CP, D], F32, kind="ExternalInput").ap()
    s_dram = nc.dram_tensor("distance_scale", [1], F32, kind="ExternalInput").ap()
    i_dram = nc.dram_tensor("identb", [128, 128], BF16, kind="ExternalInput").ap()
    o_dram = nc.dram_tensor("out", [BS, C], BF16, kind="ExternalOutput").ap()

    with tile.TileContext(nc) as tc, ExitStack() as ctx:
        singles = ctx.enter_context(tc.tile_pool(name="singles", bufs=1))
        ppool = ctx.enter_context(tc.tile_pool(name="ppool", bufs=8))
        fpool = ctx.enter_context(tc.tile_pool(name="fpool", bufs=8))
        fbpool = ctx.enter_context(tc.tile_pool(name="fbpool", bufs=8))
        sqpool = ctx.enter_context(tc.tile_pool(name="sqpool", bufs=4))
        ftpool = ctx.enter_context(tc.tile_pool(name="ftpool", bufs=4))
        opool = ctx.enter_context(tc.tile_pool(name="opool", bufs=4))
        obpool = ctx.enter_context(tc.tile_pool(name="obpool", bufs=4))
        small = ctx.enter_context(tc.tile_pool(name="small", bufs=10))
        mpsum = ctx.enter_context(tc.tile_pool(name="mpsum", bufs=3, space="PSUM"))
        tpsum = ctx.enter_context(tc.tile_pool(name="tpsum", bufs=2, space="PSUM"))

        identb = singles.tile([128, 128], BF16)
        nc.sync.dma_start(out=identb[:], in_=i_dram)

        # distance_scale -> per-partition constants: bias 2*s^2, and s^2/128
        # (dots accumulate -16*||f||*t, so scale_a = (s^2/128)*(16/||f||)
        # gives Sqrt(dots*scale_a + 2s^2) = |s|*sqrt(2-2t)).
        s_b = singles.tile([128, 1], F32)
        nc.gpsimd.dma_start(out=s_b[:], in_=s_dram.to_broadcast([128, 1]))
        s2 = singles.tile([128, 1], F32)
        nc.vector.tensor_mul(s2[:], s_b[:], s_b[:])
        two_s2 = singles.tile([128, 1], F32)
        nc.vector.tensor_scalar_mul(two_s2[:], s2[:], 2.0)
        s2_128 = singles.tile([128, 1], F32)
        nc.vector.tensor_scalar_mul(s2_128[:], s2[:], 1.0 / 128.0)
        eps = singles.tile([128, 1], F32)
        nc.vector.memset(eps[:], 1e-24)

        # ---- prototypes: normalize rows * -16, bf16, PE-transpose, fp8 ----
        pnT8 = singles.tile([128, KC, CP], F8)
        pn2q = small.tile([128, CP // 128], F32, tag="pn2q")
        pts = []
        for cb in range(CP // 128):
            pt = ppool.tile([128, D], F32, tag=f"pt{cb}")
            pts.append(pt)
            nc.sync.dma_start(out=pt[:], in_=p_dram[ts(cb, 128), :])
            psq = sqpool.tile([128, D], BF16, tag="psq")
            nc.scalar.activation(
                psq[:], pt[:], mybir.ActivationFunctionType.Square,
                accum_out=pn2q[:, cb : cb + 1])
        # ||p_c||/16 -> -16/||p_c||
        ps16 = small.tile([128, CP // 128], F32, tag="ps16")
        nc.scalar.activation(ps16[:], pn2q[:], mybir.ActivationFunctionType.Sqrt,
                             bias=eps[:], scale=1.0 / 256.0)
        pneg = small.tile([128, CP // 128], F32, tag="pneg")
        nc.vector.reciprocal(pneg[:], ps16[:])
        nc.vector.tensor_scalar_mul(pneg[:], pneg[:], -1.0)
        for cb in range(CP // 128):
            pnb = fbpool.tile([128, D], BF16, tag="pnb")
            nc.vector.tensor_scalar_mul(pnb[:], pts[cb][:], pneg[:, cb : cb + 1])
            # all 4 chunks chained into one 2KB PSUM bank as one accumulation
            # group: start=True on kc0 arms pending-zero for the whole bank
            # (ZERO_REGION_SIZE=2KB); later chunks write through clean.
            tps = tpsum.tile([128, KC, 128], BF16, tag="tps")
            for kc in range(KC):
                nc.tensor.matmul(
                    tps[:, kc, :], pnb[:, ts(kc, 128)], identb[:],
                    is_transpose=True, start=(kc == 0), stop=(kc == KC - 1),
                    skip_group_check=True)
            nc.vector.tensor_copy(out=pnT8[:, :, ts(cb, 128)], in_=tps[:])

        # ---- main loop over 64 blocks of 128 feature rows ----
        n2q = None
        scaq = None
        fbs = [None] * 4
        for ib in range(NB):
            j = ib % 4
            if j == 0:
                n2q = small.tile([128, 4], F32, tag="n2q")
                scaq = small.tile([128, 4], F32, tag="scaq")
            ft = fpool.tile([128, D], F32, tag="ft")
            nc.sync.dma_start(out=ft[:], in_=f_dram[ts(ib, 128), :])
            fb = fbpool.tile([128, D], BF16, tag="fb")
            nc.vector.tensor_copy(out=fb[:], in_=ft[:])
            fsq = sqpool.tile([128, D], BF16, tag="fsq")
            if ib % 4 == 1:
                # offload a quarter of the row-norm squares to DVE
                nc.vector.tensor_mul(fsq[:], fb[:], fb[:])
                nc.vector.reduce_sum(n2q[:, j : j + 1], fsq[:],
                                     axis=mybir.AxisListType.X)
            else:
                nc.scalar.activation(
                    fsq[:], fb[:], mybir.ActivationFunctionType.Square,
                    accum_out=n2q[:, j : j + 1])
            fbs[j] = fb
            if j == 3:
                # batched over 4 blocks: ||f||/16 -> (s^2/128)*(16/||f||)
                s16 = small.tile([128, 4], F32, tag="s16")
                nc.scalar.activation(
                    s16[:], n2q[:], mybir.ActivationFunctionType.Sqrt,
                    bias=eps[:], scale=1.0 / 256.0)
                nc.vector.reciprocal(scaq[:], s16[:])
                nc.vector.tensor_scalar_mul(scaq[:], scaq[:], s2_128[:, 0:1])
                # pass 1: transposes + fp8 copies + matmuls for all 4 blocks.
                # Emitting the sqrt/negate pass separately keeps negate(b)
                # (which depends on the full MM+sqrt chain of b) from sitting
                # in the DVE FIFO ahead of copy(b+1), which gates b+1's MMs.
                dots4 = []
                for jj in range(4):
                    tps = tpsum.tile([128, KC, 128], BF16, tag="tps")
                    for kc in range(KC):
                        nc.tensor.matmul(
                            tps[:, kc, :], fbs[jj][:, ts(kc, 128)], identb[:],
                            is_transpose=True, start=(kc == 0),
                            stop=(kc == KC - 1), skip_group_check=True)
                    fT8 = ftpool.tile([128, KC, 128], F8, tag="fT8")
                    nc.vector.tensor_copy(out=fT8[:], in_=tps[:])

                    dots = mpsum.tile([128, C], F32, tag="dots")
                    dots4.append(dots)
                    for g in range(2):
                        for lo, width in NSPLIT:
                            nc.tensor.matmul(
                                dots[:, lo : lo + width],
                                fT8[:, 2 * g : 2 * g + 2, :],
                                pnT8[:, 2 * g : 2 * g + 2, lo : lo + width],
                                start=(g == 0),
                                stop=(g == 1),
                                perf_mode=DR,
                                skip_group_check=True,
                            )
                # pass 2: sqrt + negate + store
                for jj in range(4):
                    ib2 = ib - 3 + jj
                    ot = opool.tile([128, C], BF16, tag="ot")
                    nc.scalar.activation(
                        ot[:], dots4[jj][:], mybir.ActivationFunctionType.Sqrt,
                        bias=two_s2[:], scale=scaq[:, jj : jj + 1],
                    )
                    obf = obpool.tile([128, C], BF16, tag="obf")
                    nc.vector.tensor_scalar_mul(obf[:], ot[:], -1.0)
                    nc.sync.dma_start(out=o_dram[ts(ib2, 128), :], in_=obf[:])


def build():
    nc = bacc.Bacc("TRN2", target_bir_lowering=False, debug=False,
                   num_devices=N_CORES)
    _emit(nc)
    nc.compile()
    return nc


def _ensure_ntff_hook():
    """Dev-only: restore the axon NTFF profile hook that the trimmed agent
    image's antenv package lacks, so trace=True yields real HW timings."""
    import sys
    import types

    try:
        from antenv.axon_hooks import get_axon_ntff_profile_hook  # noqa: F401
        return
    except ImportError:
        pass
    from trn_agent_boot.trn_boot import _ntff_profile_via_ctypes

    hook = _ntff_profile_via_ctypes("/opt/axon/libaxon_pjrt.so")
    mod = types.ModuleType("antenv.axon_hooks")
    mod.get_axon_ntff_profile_hook = lambda: hook
    mod.set_axon_ntff_profile_hook = lambda h: None
    sys.modules["antenv.axon_hooks"] = mod


def run(inputs, trace=False):
    if trace:
        _ensure_ntff_hook()
    import ml_dtypes

    feats = np.ascontiguousarray(np.asarray(inputs["features"], dtype=np.float32))
    protos = np.ascontiguousarray(np.asarray(inputs["prototypes"], dtype=np.float32))
    dscale = np.ascontiguousarray(np.asarray(inputs["distance_scale"], dtype=np.float32))
    protos_p = np.zeros((CP, D), dtype=np.float32)
    protos_p[:C] = protos
    ident = np.eye(128, dtype=np.float32).astype(ml_dtypes.bfloat16)
    nc = build()
    in_maps = [
        {
            "features": feats[i * BS : (i + 1) * BS],
            "prototypes": protos_p,
            "distance_scale": dscale,
            "identb": ident,
        }
        for i in range(N_CORES)
    ]
    res = run_bass_kernel_spmd(nc, in_maps, core_ids=list(range(N_CORES)),
                               trace=trace)
    out = np.concatenate(
        [np.asarray(r["out"]).astype(np.float32) for r in res.results], axis=0)
    return out, res


def kernel(**inputs) -> np.ndarray:
    out, _ = run(inputs, trace=False)
    return out


# revision 31
# speedup vs baseline: 1.0458x; 1.0458x over previous
"""IsoMaxPlus first-part kernel for TRN2 (8 NeuronCores, data-parallel on B).

out[b, c] = -|s| * sqrt(max(2 - 2 * <f_b/||f_b||, p_c/||p_c||>, 1e-12))

v3 strategy per core (B-shard of 8192 rows):
  prolog: load prototypes (host-padded to [1024, 512]) fp32, row-normalize
          and scale by -16 into bf16 (DVE), PE-transpose (identity matmul),
          DVE psum->SBUF copy casting to fp8e4: pnT8 [128, 4, 1024].
  main:   64 blocks of 128 feature rows:
          DMA f [128,512] fp32 -> DVE bf16 cast fb -> DVE
          tensor_tensor_reduce(fb*fb, row-accum) -> batched (per 4 blocks)
          ACT Sqrt + DVE recip + DVE mul give scale_a = s^2/128 * 16/||f||
          -> 4x PE bf16 transpose into half a PSUM bank -> DVE copy to SBUF
          fp8 -> 2 n-splits x 2 DoubleRow fp8 matmuls (dots = -16*||f||*t,
          fp32 psum) -> ACT Sqrt(dots*scale_a + 2s^2) psum->SBUF bf16 ->
          DVE negate -> DMA out bf16.
  host:   upcast bf16 -> fp32, concat shards.
Engine budget per block: DMA ~1.4us HBM / PE ~1.2-2us / ACT ~1.2us /
DVE ~1.4us -> target ~100-130us/core (vs 729us baseline).
"""

import numpy as np
from contextlib import ExitStack

import concourse.bass as bass
import concourse.tile as tile
from concourse import bacc, mybir
from concourse.bass import ts
from concourse.bass_utils import run_bass_kernel_spmd

N_CORES = 8
B, D, C = 65536, 512, 1000
CP = 1024                  # prototypes padded (zeros) to a multiple of 128
BS = B // N_CORES          # 8192 rows per core
NB = BS // 128             # 64 row blocks
KC = D // 128              # 4 contraction chunks
NSPLIT = ((0, 512), (512, 488))  # psum bank halves
F32 = mybir.dt.float32
BF16 = mybir.dt.bfloat16
F8 = mybir.dt.float8e4
DR = mybir.MatmulPerfMode.DoubleRow


def _emit(nc):
    f_dram = nc.dram_tensor("features", [BS, D], F32, kind="ExternalInput").ap()
    p_dram = nc.dram_tensor("prototypes", [CP, D], F32, kind="ExternalInput").ap()
    s_dram = nc.dram_tensor("distance_scale", [1], F32, kind="ExternalInput").ap()
    i_dram = nc.dram_tensor("identb", [128, 128], BF16, kind="ExternalInput").ap()
    o_dram = nc.dram_tensor("out", [BS, C], BF16, kind="ExternalOutput").ap()

    with tile.TileContext(nc) as tc, ExitStack() as ctx:
        singles = ctx.enter_context(tc.tile_pool(name="singles", bufs=1))
        ppool = ctx.enter_context(tc.tile_pool(name="ppool", bufs=8))
        fpool = ctx.enter_context(tc.tile_pool(name="fpool", bufs=8))
        fbpool = ctx.enter_context(tc.tile_pool(name="fbpool", bufs=8))
        sqpool = ctx.enter_context(tc.tile_pool(name="sqpool", bufs=4))
        ftpool = ctx.enter_context(tc.tile_pool(name="ftpool", bufs=4))
        opool = ctx.enter_context(tc.tile_pool(name="opool", bufs=4))
        obpool = ctx.enter_context(tc.tile_pool(name="obpool", bufs=4))
        small = ctx.enter_context(tc.tile_pool(name="small", bufs=10))
        mpsum = ctx.enter_context(tc.tile_pool(name="mpsum", bufs=3, space="PSUM"))
        tpsum = ctx.enter_context(tc.tile_pool(name="tpsum", bufs=2, space="PSUM"))

        identb = singles.tile([128, 128], BF16)
        nc.sync.dma_start(out=identb[:], in_=i_dram)

        # distance_scale -> per-partition constants: bias 2*s^2, and s^2/128
        # (dots accumulate -16*||f||*t, so scale_a = (s^2/128)*(16/||f||)
        # gives Sqrt(dots*scale_a + 2s^2) = |s|*sqrt(2-2t)).
        s_b = singles.tile([128, 1], F32)
        nc.gpsimd.dma_start(out=s_b[:], in_=s_dram.to_broadcast([128, 1]))
        s2 = singles.tile([128, 1], F32)
        nc.vector.tensor_mul(s2[:], s_b[:], s_b[:])
        two_s2 = singles.tile([128, 1], F32)
        nc.vector.tensor_scalar_mul(two_s2[:], s2[:], 2.0)
        s2_128 = singles.tile([128, 1], F32)
        nc.vector.tensor_scalar_mul(s2_128[:], s2[:], 1.0 / 128.0)
        eps = singles.tile([128, 1], F32)
        nc.vector.memset(eps[:], 1e-24)
        # dummy activations so both ACT table-set loads (~1.3us each) overlap
        # the first DMAs instead of stalling the first real Square/Sqrt
        warm = singles.tile([128, 1], F32)
        nc.scalar.activation(warm[:], eps[:], mybir.ActivationFunctionType.Square)
        nc.scalar.activation(warm[:], eps[:], mybir.ActivationFunctionType.Sqrt)

        # ---- prototypes: normalize rows * -16, bf16, PE-transpose, fp8 ----
        pnT8 = singles.tile([128, KC, CP], F8)
        pn2q = small.tile([128, CP // 128], F32, tag="pn2q")
        pts = []
        for cb in range(CP // 128):
            pt = ppool.tile([128, D], F32, tag=f"pt{cb}")
            pts.append(pt)
            nc.sync.dma_start(out=pt[:], in_=p_dram[ts(cb, 128), :])
            psq = sqpool.tile([128, D], BF16, tag="psq")
            nc.scalar.activation(
                psq[:], pt[:], mybir.ActivationFunctionType.Square,
                accum_out=pn2q[:, cb : cb + 1])
        # ||p_c||/16 -> -16/||p_c||
        ps16 = small.tile([128, CP // 128], F32, tag="ps16")
        nc.scalar.activation(ps16[:], pn2q[:], mybir.ActivationFunctionType.Sqrt,
                             bias=eps[:], scale=1.0 / 256.0)
        pneg = small.tile([128, CP // 128], F32, tag="pneg")
        nc.vector.reciprocal(pneg[:], ps16[:])
        nc.vector.tensor_scalar_mul(pneg[:], pneg[:], -1.0)
        for cb in range(CP // 128):
            pnb = fbpool.tile([128, D], BF16, tag="pnb")
            nc.vector.tensor_scalar_mul(pnb[:], pts[cb][:], pneg[:, cb : cb + 1])
            # all 4 chunks chained into one 2KB PSUM bank as one accumulation
            # group: start=True on kc0 arms pending-zero for the whole bank
            # (ZERO_REGION_SIZE=2KB); later chunks write through clean.
            tps = tpsum.tile([128, KC, 128], BF16, tag="tps")
            for kc in range(KC):
                nc.tensor.matmul(
                    tps[:, kc, :], pnb[:, ts(kc, 128)], identb[:],
                    is_transpose=True, start=(kc == 0), stop=(kc == KC - 1),
                    skip_group_check=True)
            nc.vector.tensor_copy(out=pnT8[:, :, ts(cb, 128)], in_=tps[:])

        # ---- main loop over 64 blocks of 128 feature rows ----
        n2q = None
        scaq = None
        fbs = [None] * 4
        for ib in range(NB):
            j = ib % 4
            if j == 0:
                n2q = small.tile([128, 4], F32, tag="n2q")
                scaq = small.tile([128, 4], F32, tag="scaq")
            ft = fpool.tile([128, D], F32, tag="ft")
            nc.sync.dma_start(out=ft[:], in_=f_dram[ts(ib, 128), :])
            fb = fbpool.tile([128, D], BF16, tag="fb")
            nc.vector.tensor_copy(out=fb[:], in_=ft[:])
            fsq = sqpool.tile([128, D], BF16, tag="fsq")
            if ib % 4 == 1 or ib % 16 == 3:
                # offload ~1/3 of the row-norm squares to DVE
                nc.vector.tensor_mul(fsq[:], fb[:], fb[:])
                nc.vector.reduce_sum(n2q[:, j : j + 1], fsq[:],
                                     axis=mybir.AxisListType.X)
            else:
                nc.scalar.activation(
                    fsq[:], fb[:], mybir.ActivationFunctionType.Square,
                    accum_out=n2q[:, j : j + 1])
            fbs[j] = fb
            if j == 3:
                # batched over 4 blocks: ||f||/16 -> (s^2/128)*(16/||f||)
                s16 = small.tile([128, 4], F32, tag="s16")
                nc.scalar.activation(
                    s16[:], n2q[:], mybir.ActivationFunctionType.Sqrt,
                    bias=eps[:], scale=1.0 / 256.0)
                nc.vector.reciprocal(scaq[:], s16[:])
                nc.vector.tensor_scalar_mul(scaq[:], scaq[:], s2_128[:, 0:1])
                # pass 1: transposes + fp8 copies + matmuls for all 4 blocks.
                # Emitting the sqrt/negate pass separately keeps negate(b)
                # (which depends on the full MM+sqrt chain of b) from sitting
                # in the DVE FIFO ahead of copy(b+1), which gates b+1's MMs.
                dots4 = []
                for jj in range(4):
                    tps = tpsum.tile([128, KC, 128], BF16, tag="tps")
                    for kc in range(KC):
                        nc.tensor.matmul(
                            tps[:, kc, :], fbs[jj][:, ts(kc, 128)], identb[:],
                            is_transpose=True, start=(kc == 0),
                            stop=(kc == KC - 1), skip_group_check=True)
                    fT8 = ftpool.tile([128, KC, 128], F8, tag="fT8")
                    # copy in halves: the g=0 matmuls only need kc 0-1, so
                    # they start after half the copy latency
                    nc.vector.tensor_copy(out=fT8[:, 0:2, :], in_=tps[:, 0:2, :])
                    nc.vector.tensor_copy(out=fT8[:, 2:4, :], in_=tps[:, 2:4, :])

                    dots = mpsum.tile([128, C], F32, tag="dots")
                    dots4.append(dots)
                    for g in range(2):
                        for lo, width in NSPLIT:
                            nc.tensor.matmul(
                                dots[:, lo : lo + width],
                                fT8[:, 2 * g : 2 * g + 2, :],
                                pnT8[:, 2 * g : 2 * g + 2, lo : lo + width],
                                start=(g == 0),
                                stop=(g == 1),
                                perf_mode=DR,
                                skip_group_check=True,
                            )
                # pass 2: sqrt + negate + store
                for jj in range(4):
                    ib2 = ib - 3 + jj
                    ot = opool.tile([128, C], BF16, tag="ot")
                    nc.scalar.activation(
                        ot[:], dots4[jj][:], mybir.ActivationFunctionType.Sqrt,
                        bias=two_s2[:], scale=scaq[:, jj : jj + 1],
                    )
                    obf = obpool.tile([128, C], BF16, tag="obf")
                    nc.vector.tensor_scalar_mul(obf[:], ot[:], -1.0)
                    nc.sync.dma_start(out=o_dram[ts(ib2, 128), :], in_=obf[:])


def build():
    nc = bacc.Bacc("TRN2", target_bir_lowering=False, debug=False,
                   num_devices=N_CORES)
    _emit(nc)
    nc.compile()
    return nc


def _ensure_ntff_hook():
    """Dev-only: restore the axon NTFF profile hook that the trimmed agent
    image's antenv package lacks, so trace=True yields real HW timings."""
    import sys
    import types

    try:
        from antenv.axon_hooks import get_axon_ntff_profile_hook  # noqa: F401
        return
    except ImportError:
        pass
    from trn_agent_boot.trn_boot import _ntff_profile_via_ctypes

    hook = _ntff_profile_via_ctypes("/opt/axon/libaxon_pjrt.so")
    mod = types.ModuleType("antenv.axon_hooks")
    mod.get_axon_ntff_profile_hook = lambda: hook
    mod.set_axon_ntff_profile_hook = lambda h: None
    sys.modules["antenv.axon_hooks"] = mod


def run(inputs, trace=False):
    if trace:
        _ensure_ntff_hook()
    import ml_dtypes

    feats = np.ascontiguousarray(np.asarray(inputs["features"], dtype=np.float32))
    protos = np.ascontiguousarray(np.asarray(inputs["prototypes"], dtype=np.float32))
    dscale = np.ascontiguousarray(np.asarray(inputs["distance_scale"], dtype=np.float32))
    protos_p = np.zeros((CP, D), dtype=np.float32)
    protos_p[:C] = protos
    ident = np.eye(128, dtype=np.float32).astype(ml_dtypes.bfloat16)
    nc = build()
    in_maps = [
        {
            "features": feats[i * BS : (i + 1) * BS],
            "prototypes": protos_p,
            "distance_scale": dscale,
            "identb": ident,
        }
        for i in range(N_CORES)
    ]
    res = run_bass_kernel_spmd(nc, in_maps, core_ids=list(range(N_CORES)),
                               trace=trace)
    out = np.concatenate(
        [np.asarray(r["out"]).astype(np.float32) for r in res.results], axis=0)
    return out, res


def kernel(**inputs) -> np.ndarray:
    out, _ = run(inputs, trace=False)
    return out
